# revision 84
# baseline (speedup 1.0000x reference)
"""MAB (Set-Transformer multihead attention block) Trainium2 Bass kernel.

Reference math (fp32):
  Q = q @ Wq.T + bq ; K = k @ Wk.T + bk ; V = k @ Wv.T + bv    [B,N,256]
  per head h (8 heads x 32): s = Qh @ Kh.T / 16 ; a = softmax(s)
  Oh = Qh + a @ Vh ; o = concat(Oh) ; o = LN0(o) ; o = o + relu(o @ Wo.T + bo)
  out = LN1(o)

Sharding: 8 cores = (batch b in 0..3, query-half in 0..1). Each core handles
1024 queries x 2048 keys of one batch; rows are fully independent through
the whole block (LN/FFN are per-row), so there are no collectives.

Per-core design (~146us on the TimelineSim cost model vs ~187us for the
previous ACT-only-exp version).  The dominant cost is the softmax exp
(16.8M exps/core, ~110us on ACT alone), so the exp stream is SPLIT
between ACT (hardware Exp) and DVE (Schraudolph bit-trick exp:
i16 = trunc(x*184.665/16 + 16251), bitcast to bf16; the ~3% elementwise
error largely cancels in the softmax ratio and averages out in PV --
measured end-to-end error 7.6e-3 vs the 2e-2 gate).  The per-(head,group)
engine assignment (EXP_SCHED) is tuned so ACT takes more batches exactly
where DVE runs tail chains.  Everything else is arranged to minimize the
two exp engines' remaining work and keep the pipeline dense:
  - scores via fp8e4m3 K_T/Q_T with MatmulPerfMode.DoubleRow: the 32-dim
    head contraction is packed as 32 partitions x 2-interleave (second
    interleave slot zeroed once by GPSIMD), halving PE score time to
    0.5 cycles/row.  fp8 quantization adds only ~0.3% attention-weight
    noise since scores are divided by 16.
  - one unified PSUM layout: 3 x 2-bank rotating slots shared by score
    tiles [P,4,256], projection chunks and transposes, plus 2 banks for
    PV.  3 slots keep the scores->exp->PV pipeline 3 deep so neither exp
    engine ever waits on PSUM recycling.
  - PV with the attention tile stationary: out[q, 33] = attn.T @ V_aug
    streams 33 columns; the ones column accumulates the softmax
    denominator for free.  PV matmuls lag up to 7 exp groups behind the
    score stream.
  - feature-axis biases (bv, bq, bo') fold into the PE accumulation as
    ones-row outer products; partition-axis biases (K/Q proj) ride the
    PSUM->SBUF evacuation (ACT Identity+bias or DVE tensor_scalar_add,
    chosen per call for load balance).  Input/weight transposes share one
    [P,2,256] copy per token-tile pair.
  - LayerNorm via bn_stats/bn_aggr, one shared quake-rsqrt Newton chain
    (1 iteration) for both query tiles, normalize split DVE/GPSIMD.
    b0 drops out exactly (LN1's mean subtraction cancels constants), g0
    folds into Wo' = Wo diag(g0) and bo' = bo + Wo @ b0 on-chip.
  - the LN affine, residual adds and FFN adds run on the otherwise-idle
    GPSIMD engine (SBUF-only operands; GPSIMD never touches PSUM).
  - post-attention datapath is bf16 (2x DVE throughput on packed 16-bit
    tensor ops, 1 cycle/row PE transposes, bf16 FFN matmul).
  - mid-stream tails are split into 5 pieces emitted at spread
    (head,group) callback slots inside the next q-group's attention so
    serial LN chains never stall the in-order DVE queue; the final
    q-group runs a per-query-tile pipelined tail across all engines.
  - input DMAs are split by criticality (half-weights, k/q quarters) so
    the first score fires ~5.5us in.
  HW constraints honored: one ACT table set (exp_and_others) covers
  Exp/Identity/Relu/Copy so no table reloads; fp32r matmul inputs only
  come from f32r-rounded producers; GPSIMD touches only SBUF.
"""

import os
import sys
from contextlib import ExitStack

import numpy as np

for _p in ("/opt/trn_rl_repo", "/root/.axon_site/_ro/trn_rl_repo"):
    if os.path.isdir(_p) and _p not in sys.path:
        sys.path.insert(0, _p)

import concourse.bass as bass  # noqa: E402
import concourse.tile as tile  # noqa: E402
from concourse import bacc, mybir  # noqa: E402
from concourse.masks import make_identity  # noqa: E402

F32 = mybir.dt.float32
FP8 = mybir.dt.float8e4
F32R = mybir.dt.float32r
BF16 = mybir.dt.bfloat16
I16 = mybir.dt.int16
I32 = mybir.dt.int32
P = 128
EPS = 1e-5

AF = mybir.ActivationFunctionType
OP = mybir.AluOpType
AX = mybir.AxisListType

# Schraudolph exp in bf16 bit-space, including the 1/16 score scale and
# +0.5 for the executor's truncating float->int16 conversion.
SCHRAUD_A = (128.0 / float(np.log(2.0))) / 16.0
SCHRAUD_B = 127.0 * 128.0 + 0.5 - 5.5

# Engine schedule knobs (tuned against TimelineSim).
EXP_PAT = "AADADAD"
EXP_SCHED0 = ("AAAD" "AADA" "ADAD" "ADAD" "ADDA" "ADDA" "DADD" "ADDA")
# per-(h,g) engine map for qg>0: ACT-heavy where DVE runs tail chains
EXP_SCHED = ("ADAD" "AADA" "ADAA" "DAAD" "ADAD" "ADAD" "ADAD" "AADA")      # per exp batch: A=ACT exp, D=DVE schraudolph
KQ_ENG = "a"           # K_T/Q_T projection bias-copies
V_PAT = "ad"           # V_aug copies rotate over this pattern
QNAT_ENG = "a"         # Q_nat copies
TRANS_PAT = "da"       # input-transpose copies rotate over this
RELU_ENG = "a"
FINAL_TAIL = True
NEWTON_ITERS = 1


class Cfg:
    def __init__(self, NQ=1024, NK=2048, D=256, H=8):
        self.NQ, self.NK, self.D, self.H = NQ, NK, D, H
        self.HD = D // H            # 32
        self.DO = D // P            # 2
        self.QT = NQ // P           # 8
        self.KT = NK // P           # 16
        self.QG = NQ // 256         # q-groups of 256
        self.KG = 4                 # k-tiles per exp group
        assert self.HD == 32 and self.DO == 2 and self.KT % self.KG == 0


def _emit(nc: bass.Bass, tc: tile.TileContext, ctx: ExitStack, io: dict, cfg: Cfg):
    NQ, NK, D, H = cfg.NQ, cfg.NK, cfg.D, cfg.H
    DO, QT, KT, QG, KG = cfg.DO, cfg.QT, cfg.KT, cfg.QG, cfg.KG
    NG = KT // KG                   # exp groups per (qg, h)

    def r(ap):                      # fp32 -> fp32r view for fast matmuls
        return ap if ap.dtype == F32R else ap.bitcast(F32R)

    const = ctx.enter_context(tc.tile_pool(name="const", bufs=1))
    persist = ctx.enter_context(tc.tile_pool(name="persist", bufs=1))

    ident = const.tile([P, P], F32)
    make_identity(nc, ident)
    ident_b = const.tile([P, P], BF16)
    make_identity(nc, ident_b)
    magic2 = const.tile([P, 2], I32)   # quake rsqrt seed (paired)
    nc.vector.memset(magic2, 0x5F3759DF)
    ones_b = const.tile([1, P], BF16)
    nc.vector.memset(ones_b, 1.0)

    # ---- head phase: DMAs ordered by criticality, chunked for pipelining.
    w_sb = {}

    def w_dma(name):
        t = const.tile([P, DO, D], F32, name=f"{name}_nat")
        nc.sync.dma_start(t, io[name][:].rearrange("(o p) f -> p o f", p=P))
        w_sb[name] = t

    q_sb = const.tile([P, QT, D], F32, name="q_nat")
    k_sb = const.tile([P, KT, D], F32, name="k_nat")
    q_dram = io["q"][:].rearrange("(t p) d -> p t d", p=P)
    k_dram = io["k"][:].rearrange("(t p) d -> p t d", p=P)
    KH, QH = KT // 2, QT // 2
    KQ = KT // 4

    def vec_pm(name):
        t = const.tile([P, DO], F32, name=f"{name}_pm")
        nc.sync.dma_start(t, io[name][:].rearrange("(o p) -> p o", p=P))
        return t

    def row_dma(name):
        t = const.tile([1, D], F32, name=f"{name}_row")
        nc.sync.dma_start(t, io[name][:].rearrange("(o d) -> o d", o=1))
        return t

    def w_dma_half(name, o):
        if name not in w_sb:
            w_sb[name] = const.tile([P, DO, D], F32, name=f"{name}_nat")
        nc.sync.dma_start(
            w_sb[name][:, o], io[name][:].rearrange("(o p) f -> p o f", p=P)[:, o])

    QQ = QT // 4
    w_dma_half("Wk", 0)
    nc.sync.dma_start(k_sb[:, :KQ], k_dram[:, :KQ])
    nc.gpsimd.dma_start(q_sb[:, :QQ], q_dram[:, :QQ])
    w_dma_half("Wq", 0)
    bq_pm, bk_pm = vec_pm("bq"), vec_pm("bk")
    nc.gpsimd.dma_start(q_sb[:, QQ:QH], q_dram[:, QQ:QH])
    nc.sync.dma_start(k_sb[:, KQ:2 * KQ], k_dram[:, KQ:2 * KQ])
    w_dma_half("Wk", 1)
    w_dma_half("Wq", 1)
    nc.sync.dma_start(k_sb[:, 2 * KQ:3 * KQ], k_dram[:, 2 * KQ:3 * KQ])
    nc.sync.dma_start(k_sb[:, 3 * KQ:], k_dram[:, 3 * KQ:])
    nc.gpsimd.dma_start(q_sb[:, QH:], q_dram[:, QH:])
    w_dma("Wv")
    w_dma("Wo")
    bv_row, bq_row, bo_row = row_dma("bv"), row_dma("bq"), row_dma("bo")
    b0_pm, g0_pm = vec_pm("b0"), vec_pm("g0")
    bv_b = const.tile([1, D], BF16, name="bv_b")
    nc.vector.tensor_copy(bv_b, bv_row)
    bq_b = const.tile([1, D], BF16, name="bq_b")
    nc.vector.tensor_copy(bq_b, bq_row)

    # row vectors broadcast across partitions: B[p, d] = vec[d]
    def bcast(name):
        row = row_dma(name)
        t = const.tile([P, D], F32, name=f"{name}_b")
        nc.gpsimd.partition_broadcast(t, row)
        return t

    G0b = bcast("g0")
    G1b, B1b = bcast("g1"), bcast("b1")
    G0bb = const.tile([P, D], BF16, name="G0bb")
    nc.vector.tensor_copy(G0bb, G0b)

    # ---- transposes + projections, emission-ordered along the k0/q0 chain
    wT = {}

    q_T = const.tile([P, DO, NQ], F32R, name="q_T")
    k_T = const.tile([P, DO, NK], F32R, name="k_T")

    Q_T = persist.tile([P, DO, 2, NQ], FP8, name="Q_T")
    K_T = persist.tile([P, DO, 2, NK], FP8, name="K_T")
    nc.gpsimd.memset(K_T[:, :, 1, :], 0.0)
    nc.gpsimd.memset(Q_T[:, :, 1, :], 0.0)
    Q_nat = persist.tile([P, QT, D], BF16, name="Q_nat")
    V_aug = persist.tile([P, KT, H, 33], BF16, name="V_aug")
    nc.vector.memset(V_aug[:, :, :, 32], 1.0)
    bo2_row = const.tile([1, D], BF16, name="bo2_row")

    score_p = ctx.enter_context(tc.tile_pool(name="score_p", bufs=3, space="PSUM"))
    pv_p = ctx.enter_context(tc.tile_pool(name="pv_p", bufs=1, space="PSUM"))
    attn_p = ctx.enter_context(tc.tile_pool(name="attn_p", bufs=12))
    work = ctx.enter_context(tc.tile_pool(name="work", bufs=2))

    def proj_ps():
        return score_p.tile([P, 512], F32, name="pp", tag="s")

    def w_transpose(name, eng, dtype=F32R, os_=(0, 1)):
        if name in wT:
            t = wT[name]
        else:
            t = persist.tile([P, DO, D], dtype, name=f"{name}T")
            wT[name] = t
        for o in os_:
            for fo in range(DO):
                ps = proj_ps()[:, :P]
                nc.tensor.transpose(ps,
                                    w_sb[name][:, o, fo * P:(fo + 1) * P],
                                    ident)
                if eng is nc.scalar:
                    eng.copy(t[:, fo, o * P:(o + 1) * P], ps)
                else:
                    eng.tensor_copy(t[:, fo, o * P:(o + 1) * P], ps)

    trans_tick = [0]

    def in_transpose(src, dst, t0, t1, step=2):
        # `step` token tiles (2*step transposes) share one [P,2,step*128]
        # copy through a full PSUM slot
        for t in range(t0, t1, step):
            tn = min(t + step, t1) - t
            ps = score_p.tile([P, 1024], F32, name="tps", tag="s")
            for i in range(tn):
                for o in range(DO):
                    nc.tensor.transpose(
                        ps[:, (o * tn + i) * P:(o * tn + i + 1) * P],
                        src[:, t + i, o * P:(o + 1) * P], ident)
            e = TRANS_PAT[trans_tick[0] % len(TRANS_PAT)]
            trans_tick[0] += 1
            dstv = dst[:, :, t * P:(t + tn) * P]
            srcv = ps[:, :tn * 256].rearrange("p (o tc) -> p o tc", o=2)
            if e == "a":
                nc.scalar.copy(dstv, srcv)
            else:
                nc.vector.tensor_copy(dstv, srcv)

    def proj_T(w, src, dst, b_pm, o, c0, cn, step=512, eng=None):
        eng = eng or KQ_ENG
        for c in range(c0, c0 + cn, step):
            cw = min(step, c0 + cn - c)
            ps = proj_ps()[:, :cw]
            for ki in range(DO):
                nc.tensor.matmul(
                    ps, lhsT=r(wT[w][:, ki, o * P:(o + 1) * P]),
                    rhs=r(src[:, ki, c:c + cw]),
                    start=(ki == 0), stop=(ki == DO - 1))
            if eng == "a":
                nc.scalar.activation(dst[:, o, 0, c:c + cw], ps, AF.Identity,
                                     bias=b_pm[:, o:o + 1])
            else:
                nc.vector.tensor_scalar_add(dst[:, o, 0, c:c + cw], ps,
                                            b_pm[:, o:o + 1])

    v_tick = [0]

    def v_proj(t0, t1):
        for t in range(t0, t1, 2):
            ps = proj_ps()
            for i in range(2):
                for ki in range(DO):
                    nc.tensor.matmul(
                        ps[:, i * D:(i + 1) * D],
                        lhsT=r(k_T[:, ki, (t + i) * P:(t + i + 1) * P]),
                        rhs=r(wT["Wv"][:, ki, :]),
                        start=(ki == 0), stop=False)
                nc.tensor.matmul(ps[:, i * D:(i + 1) * D], lhsT=ones_b,
                                 rhs=bv_b, start=False, stop=True,
                                 skip_group_check=True)
            e = V_PAT[v_tick[0] % len(V_PAT)]
            v_tick[0] += 1
            dstv = V_aug[:, t:t + 2, :, :32]
            srcv = ps.rearrange("p (t h w) -> p t h w", t=2, h=H)
            if e == "a":
                nc.scalar.copy(dstv, srcv)
            else:
                nc.vector.tensor_copy(dstv, srcv)

    def qnat_proj(t0, t1):
        for t in range(t0, t1, 2):
            ps = proj_ps()
            for i in range(2):
                for ki in range(DO):
                    nc.tensor.matmul(
                        ps[:, i * D:(i + 1) * D],
                        lhsT=r(q_T[:, ki, (t + i) * P:(t + i + 1) * P]),
                        rhs=r(wT["Wq"][:, ki, :]),
                        start=(ki == 0), stop=False)
                nc.tensor.matmul(ps[:, i * D:(i + 1) * D], lhsT=ones_b,
                                 rhs=bq_b, start=False, stop=True,
                                 skip_group_check=True)
            if QNAT_ENG == "a":
                nc.scalar.copy(Q_nat[:, t:t + 2, :], ps)
            else:
                nc.vector.tensor_copy(Q_nat[:, t:t + 2, :], ps)

    def wo_finish():
        # bo' = bo + Wo @ b0 (uses unfolded WoT), then fold g0 into WoT
        wTo = wT["Wo"]
        b0b = const.tile([P, DO], BF16, name="b0b")
        nc.vector.tensor_copy(b0b, b0_pm)
        ps = proj_ps()[:1, :D]
        for ki in range(DO):
            nc.tensor.matmul(ps, lhsT=b0b[:, ki:ki + 1],
                             rhs=wTo[:, ki, :],
                             start=(ki == 0), stop=(ki == DO - 1))
        nc.vector.tensor_tensor(bo2_row, ps, bo_row, OP.add)
        for fo in range(DO):
            nc.vector.tensor_scalar_mul(wTo[:, fo, :], wTo[:, fo, :],
                                        g0_pm[:, fo:fo + 1])

    # critical prefix: exactly what the first exp groups need, k-quarter
    # by k-quarter, so the first exp fires as early as possible.
    w_transpose("Wk", nc.scalar, os_=(0,))
    in_transpose(k_sb, k_T, 0, KQ)
    proj_T("Wk", k_T, K_T, bk_pm, 0, 0, 512, eng="d")  # K o=0 kt0-3
    w_transpose("Wq", nc.scalar, os_=(0,))
    in_transpose(q_sb, q_T, 0, QQ)
    proj_T("Wq", q_T, Q_T, bq_pm, 0, 0, 256, step=256,
           eng="d")                                     # Q o=0 qg0 cols
    in_transpose(q_sb, q_T, QQ, QH)
    proj_T("Wq", q_T, Q_T, bq_pm, 0, 256, 256, step=256, eng="d")
    in_transpose(k_sb, k_T, KQ, 2 * KQ)
    proj_T("Wk", k_T, K_T, bk_pm, 0, 512, 512, eng="d")  # K o=0 kt4-7

    # late phase-0 work spread across qg0's attention groups (h, g):
    # each chunk lands just before the first scores that need it.
    qg0_cbs = {
        (0, 2): [lambda: in_transpose(k_sb, k_T, 2 * KQ, 3 * KQ),
                 lambda: proj_T("Wk", k_T, K_T, bk_pm, 0, 1024, 512)],
        (0, 3): [lambda: in_transpose(k_sb, k_T, 3 * KQ, KT),
                 lambda: proj_T("Wk", k_T, K_T, bk_pm, 0, 1536, 512),
                 lambda: w_transpose("Wv", nc.vector)],
        (1, 0): [lambda: v_proj(0, 2)],
        (1, 1): [lambda: v_proj(2, 4)],
        (1, 2): [lambda: v_proj(4, 6)],
        (1, 3): [lambda: v_proj(6, 8)],
        (2, 0): [lambda: v_proj(8, 10)],
        (2, 1): [lambda: v_proj(10, 12)],
        (2, 2): [lambda: v_proj(12, 14),
                 lambda: w_transpose("Wk", nc.scalar, os_=(1,))],
        (2, 3): [lambda: v_proj(14, 16),
                 lambda: w_transpose("Wq", nc.scalar, os_=(1,))],
        (3, 0): [lambda: proj_T("Wk", k_T, K_T, bk_pm, 1, 0, 512)],
        (3, 1): [lambda: proj_T("Wk", k_T, K_T, bk_pm, 1, 512, 512)],
        (3, 2): [lambda: proj_T("Wq", q_T, Q_T, bq_pm, 1, 0, 512)],
        (3, 3): [lambda: proj_T("Wk", k_T, K_T, bk_pm, 1, 1024, 512)],
        (4, 0): [lambda: proj_T("Wk", k_T, K_T, bk_pm, 1, 1536, 512)],
        (4, 1): [lambda: qnat_proj(0, 2)],
    }
    # work only later q-groups need, spread into their attention windows
    later_cbs = {
        1: {(2, 0): [lambda: in_transpose(q_sb, q_T, QH, QT),
                     lambda: w_transpose("Wo", nc.vector, dtype=BF16)],
            (2, 1): [lambda: wo_finish()],
            (2, 2): [lambda: proj_T("Wq", q_T, Q_T, bq_pm, 0,
                                    QH * P, QH * P)],
            (2, 3): [lambda: proj_T("Wq", q_T, Q_T, bq_pm, 1, 512, 512)],
            (3, 2): [lambda: qnat_proj(2, 4)]},
        2: {(2, 0): [lambda: qnat_proj(4, 6)]},
        3: {(2, 0): [lambda: qnat_proj(6, 8)]},
    }

    O_t = [None] * QG
    X0_t = [None] * QG
    X0m_t = [None] * QG
    exp_tick = [0]

    def emit_exp(at, ps, h=0, g=0, qg=0):
        if qg > 0:
            e = EXP_SCHED[h * 4 + g]
        else:
            e = EXP_PAT[exp_tick[0] % len(EXP_PAT)]
        exp_tick[0] += 1
        if e == "A":
            nc.scalar.activation(at, ps, AF.Exp, scale=1.0 / 16.0)
        else:
            nc.vector.tensor_scalar(at.bitcast(I16), ps,
                                    SCHRAUD_A, SCHRAUD_B, OP.mult, OP.add)

    out_nat = io["out"][:].rearrange("(t p) d -> p t d", p=P)

    def emit_attn(qg, cbs, finish_prev=None, final_tail=False):
        """Emit one q-group's attention.  cbs[(h, g)] is a list of thunks
        emitted just before that group's score matmuls.  PV matmuls lag
        several exp groups behind the score/exp stream so the in-order
        PE queue never blocks the exp pipeline.  finish_prev (the
        previous group's final PV flush + normalize) is emitted right
        after this group's first exp so the boundary never bubbles."""
        qsl = slice(qg * 256, (qg + 1) * 256)
        pvps = pv_p.tile([P, 2, H, 64], F32, tag="pv")
        pend = []   # [(at_tile, h, g)] not yet matmul'd against V

        def flush_pv(n):
            while len(pend) > n:
                at, h, g = pend.pop(0)
                for j in range(KG):
                    kt = g * KG + j
                    for qt in range(2):
                        nc.tensor.matmul(
                            pvps[:, qt, h, :33],
                            lhsT=at[:, j, qt * P:(qt + 1) * P],
                            rhs=V_aug[:, kt, h, :],
                            start=(kt == 0), stop=(kt == KT - 1),
                            skip_group_check=True)

        def finish():
            flush_pv(0)
            # normalize + residual: O = pv/denom + Q  (denom = col 32)
            O = work.tile([P, 2, D], BF16, tag="O")
            O_t[qg] = O
            rec = work.tile([P, 2, H], F32, tag="rec")
            nc.vector.reciprocal(rec, pvps[:, :, :, 32])
            for qt in range(2):
                qa = qg * 2 + qt
                Ov = O[:, qt, :].rearrange("p (h w) -> p h w", h=H)
                rb = rec[:, qt, :].unsqueeze(2).to_broadcast([P, H, 32])
                nc.vector.tensor_tensor(Ov, pvps[:, qt, :, :32], rb, OP.mult)
                nc.vector.tensor_tensor(O[:, qt, :], O[:, qt, :],
                                         Q_nat[:, qa, :], OP.add)
            if not final_tail:
                return
            # final q-group: shared-newton, per-qt pipelined tail
            fst = work.tile([P, 2, 2], F32, tag="fst", name="fst")
            fy = work.tile([P, 2], F32, tag="fy", name="fy")
            tail_stats(O, fst, fy, "fa", eng=nc.vector)
            X0 = work.tile([P, 2, D], BF16, tag="fX0", name="fX0")
            X1 = work.tile([P, 2, D], BF16, tag="fX1", name="fX1")
            for qt in range(2):
                nc.vector.scalar_tensor_tensor(
                    X0[:, qt, :], in0=O[:, qt, :], scalar=fst[:, qt, 0:1],
                    in1=fy[:, qt:qt + 1].to_broadcast([P, D]),
                    op0=OP.subtract, op1=OP.mult)
                X0mq = work.tile([P, D], BF16, tag=f"fXm{qt}", name="X0mq")
                meng = nc.gpsimd if qt == 0 else nc.vector
                meng.tensor_tensor(X0mq, X0[:, qt, :], G0b, OP.mult)
                psb = score_p.tile([P, DO, P], BF16, name="fsb", tag="s")
                for o in range(DO):
                    nc.tensor.transpose(psb[:, o, :],
                                        X0[:, qt, o * P:(o + 1) * P],
                                        ident_b)
                X0Tq = work.tile([P, DO, P], BF16, tag=f"fXT{qt}", name="X0Tq")
                nc.vector.tensor_copy(X0Tq, psb)
                ps = proj_ps()[:, :D]
                for ki in range(DO):
                    nc.tensor.matmul(ps, lhsT=X0Tq[:, ki, :],
                                     rhs=wT["Wo"][:, ki, :],
                                     start=(ki == 0), stop=False)
                nc.tensor.matmul(ps, lhsT=ones_b, rhs=bo2_row,
                                 start=False, stop=True,
                                 skip_group_check=True)
                h1q = work.tile([P, D], BF16, tag=f"fh1{qt}", name="h1q")
                nc.scalar.activation(h1q, ps, AF.Relu)
                meng.tensor_tensor(X1[:, qt, :], h1q, X0mq, OP.add)
            fst2 = work.tile([P, 2, 2], F32, tag="fst2", name="fst2")
            fy2 = work.tile([P, 2], F32, tag="fy2", name="fy2")
            tail_stats(X1, fst2, fy2, "fb", eng=nc.vector)
            X2 = work.tile([P, 2, D], BF16, tag="fX2", name="fX2")
            tail_apply(X1, fst2, fy2, X2, tag="f1", pool_qt1=False)
            for qt in range(2):
                X3q = work.tile([P, D], F32, tag=f"fX3{qt}", name="X3q")
                nc.vector.tensor_tensor(X3q, X2[:, qt, :], G1b, OP.mult)
                nc.vector.tensor_tensor(X3q, X3q, B1b, OP.add)
                nc.sync.dma_start(out_nat[:, qg * 2 + qt, :], X3q)

        for h in range(H):
            o, m = divmod(h, 4)
            hsl = slice(32 * m, 32 * m + 32)
            for g in range(NG):
                for cb in cbs.get((h, g), ()):
                    cb()
                flush_pv(9 if h < 6 else (2 if not final_tail or h < 7
                                          else 1))
                ps = score_p.tile([P, KG, 256], F32, tag="s")
                for j in range(KG):
                    kt = g * KG + j
                    nc.tensor.matmul(
                        ps[:, j, :],
                        lhsT=K_T[hsl, o, :, kt * P:(kt + 1) * P],
                        rhs=Q_T[hsl, o, :, qsl],
                        start=True, stop=True,
                        perf_mode=mybir.MatmulPerfMode.DoubleRow,
                        tile_position=(32 * m, 0))
                at = attn_p.tile([P, KG, 256], BF16, tag="at")
                emit_exp(at, ps, h, g, qg)
                pend.append((at, h, g))
                if finish_prev is not None:
                    finish_prev()
                    finish_prev = None
        return finish

    def tail_stats(X, st2, y, tag, eng=None):
        """LN stats for both qt of X [P,2,D]: bn_stats/aggr per qt, one
        shared quake-rsqrt Newton chain on [P,2] -> y (newton on `eng`,
        Pool for mid-stream tails so the DVE queue never stalls)."""
        eng = eng or nc.vector
        for qt in range(2):
            st6 = work.tile([P, 6], F32, tag=f"s6{tag}{qt}")
            nc.vector.bn_stats(st6, X[:, qt, :])
            nc.vector.bn_aggr(st2[:, qt, :], st6)
        v = work.tile([P, 2], F32, tag=f"v{tag}")
        eng.tensor_scalar_add(v, st2[:, :, 1], EPS)
        iw = work.tile([P, 2], I32, tag=f"qi{tag}")
        eng.tensor_scalar(iw, v.bitcast(I32), 1, None,
                          OP.arith_shift_right)
        eng.tensor_tensor(iw, magic2, iw, OP.subtract)
        t = work.tile([P, 2], F32, tag=f"qt{tag}")
        yv = y.bitcast(I32)
        eng.tensor_copy(yv, iw)
        yf = y
        for _ in range(NEWTON_ITERS):
            eng.tensor_tensor(t, yf, yf, OP.mult)
            eng.tensor_tensor(t, t, v, OP.mult)
            eng.tensor_scalar(t, t, -0.5, 1.5, OP.mult, OP.add)
            eng.tensor_tensor(yf, yf, t, OP.mult)

    def tail_apply(X, st2, y, dst, tag="ta", pool_qt1=True):
        # qt0 on DVE (1 stt); qt1 on Pool (2 tensor_tensor) in parallel for
        # mid-stream tails, on DVE for the latency-critical final tail
        nc.vector.scalar_tensor_tensor(
            dst[:, 0, :], in0=X[:, 0, :], scalar=st2[:, 0, 0:1],
            in1=y[:, 0:1].to_broadcast([P, D]),
            op0=OP.subtract, op1=OP.mult)
        if pool_qt1:
            tp = work.tile([P, D], F32, tag=f"tp{tag}", name="tp")
            nc.gpsimd.tensor_tensor(tp, X[:, 1, :],
                                    st2[:, 1, 0:1].to_broadcast([P, D]),
                                    OP.subtract)
            nc.gpsimd.tensor_tensor(dst[:, 1, :], tp,
                                    y[:, 1:2].to_broadcast([P, D]), OP.mult)
        else:
            nc.vector.scalar_tensor_tensor(
                dst[:, 1, :], in0=X[:, 1, :], scalar=st2[:, 1, 0:1],
                in1=y[:, 1:2].to_broadcast([P, D]),
                op0=OP.subtract, op1=OP.mult)

    st_t, y_t, st2_t, y2_t, X1_t = {}, {}, {}, {}, {}

    def tail_a_stats(qg):
        st_t[qg] = work.tile([P, 2, 2], F32, tag="ast", name="ast")
        y_t[qg] = work.tile([P, 2], F32, tag="ay", name="ay")
        tail_stats(O_t[qg], st_t[qg], y_t[qg], "a")

    def tail_a_apply(qg):
        X0 = work.tile([P, 2, D], BF16, tag="X0")
        X0_t[qg] = X0
        tail_apply(O_t[qg], st_t[qg], y_t[qg], X0)
        X0m = work.tile([P, 2, D], BF16, tag="X0m")
        X0m_t[qg] = X0m
        nc.gpsimd.tensor_tensor(X0m, X0,
                                G0b.unsqueeze(1).to_broadcast([P, 2, D]),
                                OP.mult)

    def tail_b1(qg):
        X0 = X0_t[qg]
        X0T = work.tile([P, 2, DO, P], BF16, tag="X0T")
        X1 = work.tile([P, 2, D], BF16, tag="X1")
        X1_t[qg] = X1
        for qt in range(2):
            psb = score_p.tile([P, DO, P], BF16, name="psb", tag="s")
            for o in range(DO):
                nc.tensor.transpose(psb[:, o, :],
                                    X0[:, qt, o * P:(o + 1) * P], ident_b)
            nc.vector.tensor_copy(X0T[:, qt], psb)
            ps = proj_ps()[:, :D]
            for ki in range(DO):
                nc.tensor.matmul(
                    ps, lhsT=X0T[:, qt, ki, :], rhs=wT["Wo"][:, ki, :],
                    start=(ki == 0), stop=False)
            nc.tensor.matmul(ps, lhsT=ones_b, rhs=bo2_row,
                             start=False, stop=True, skip_group_check=True)
            h1 = work.tile([P, D], BF16, tag=f"ffh{qt}")
            if RELU_ENG == "a":
                nc.scalar.activation(h1, ps, AF.Relu)
            else:
                nc.vector.tensor_scalar_max(h1, ps, 0.0)
            nc.vector.tensor_tensor(X1[:, qt, :], h1, X0m_t[qg][:, qt, :],
                                     OP.add)

    def tail_b2_stats(qg):
        st2_t[qg] = work.tile([P, 2, 2], F32, tag="bst", name="bst")
        y2_t[qg] = work.tile([P, 2], F32, tag="by", name="by")
        tail_stats(X1_t[qg], st2_t[qg], y2_t[qg], "b")

    def tail_b2_apply(qg):
        X2 = work.tile([P, 2, D], BF16, tag="X2")
        X3 = work.tile([P, 2, D], F32, tag="X3")
        tail_apply(X1_t[qg], st2_t[qg], y2_t[qg], X2)
        for qt in range(2):
            nc.gpsimd.tensor_tensor(X3[:, qt, :], X2[:, qt, :], G1b, OP.mult)
            nc.gpsimd.tensor_tensor(X3[:, qt, :], X3[:, qt, :], B1b, OP.add)
            nc.sync.dma_start(out_nat[:, qg * 2 + qt, :], X3[:, qt, :])

    fin = None
    for qg in range(QG):
        if qg == 0:
            cbs = qg0_cbs
        else:
            cbs = {k: list(v) for k, v in later_cbs.get(qg, {}).items()}
            g = qg - 1
            for slot, fns in (((1, 0), tail_a_stats), ((1, 2), tail_a_apply),
                              ((3, 0), tail_b1), ((3, 1), tail_b2_stats),
                              ((3, 3), tail_b2_apply)):
                cbs.setdefault(slot, []).append(
                    (lambda f, gg: lambda: f(gg))(fns, g))
        fin = emit_attn(qg, cbs, fin,
                        final_tail=(FINAL_TAIL and qg == QG - 1))
    fin()


def build(cfg: Cfg) -> bass.Bass:
    nc = bacc.Bacc("TRN2")
    io = {}
    for name, shape in (
        ("q", [cfg.NQ, cfg.D]), ("k", [cfg.NK, cfg.D]),
        ("Wq", [cfg.D, cfg.D]), ("Wk", [cfg.D, cfg.D]),
        ("Wv", [cfg.D, cfg.D]), ("Wo", [cfg.D, cfg.D]),
        ("bq", [cfg.D]), ("bk", [cfg.D]), ("bv", [cfg.D]), ("bo", [cfg.D]),
        ("g0", [cfg.D]), ("b0", [cfg.D]), ("g1", [cfg.D]), ("b1", [cfg.D]),
    ):
        io[name] = nc.dram_tensor(name, shape, F32, kind="ExternalInput")
    io["out"] = nc.dram_tensor("out", [cfg.NQ, cfg.D], F32, kind="ExternalOutput")

    with tile.TileContext(nc) as tc:
        with ExitStack() as ctx:
            _emit(nc, tc, ctx, io, cfg)
    nc.compile()
    return nc


_CACHE = {}


def _get_nc(key, cfg):
    if key not in _CACHE:
        _CACHE[key] = build(cfg)
    return _CACHE[key]


def kernel(q, k, Wq, bq, Wk, bk, Wv, bv, Wo, bo, g0, b0, g1, b1, _trace=False):
    from concourse.bass_utils import run_bass_kernel_spmd

    B, Nq, D = q.shape
    Nk = k.shape[1]
    n_cores = 8
    halves = n_cores // B
    nq_c = Nq // halves
    cfg = Cfg(NQ=nq_c, NK=Nk, D=D)
    nc = _get_nc((nq_c, Nk, D), cfg)

    shared = dict(Wq=Wq, bq=bq, Wk=Wk, bk=bk, Wv=Wv, bv=bv, Wo=Wo, bo=bo,
                  g0=g0, b0=b0, g1=g1, b1=b1)
    shared = {n: np.ascontiguousarray(v, dtype=np.float32)
              for n, v in shared.items()}
    in_maps = []
    for c in range(n_cores):
        b, hf = c // halves, c % halves
        m = dict(shared)
        m["q"] = np.ascontiguousarray(q[b, hf * nq_c:(hf + 1) * nq_c], np.float32)
        m["k"] = np.ascontiguousarray(k[b], np.float32)
        in_maps.append(m)

    res = run_bass_kernel_spmd(nc, in_maps, core_ids=list(range(n_cores)),
                               trace=_trace)
    out = np.empty((B, Nq, D), np.float32)
    for c in range(n_cores):
        b, hf = c // halves, c % halves
        out[b, hf * nq_c:(hf + 1) * nq_c] = res.results[c]["out"]
    if _trace:
        return out, res
    return out
